# revision 1
# baseline (speedup 1.0000x reference)
"""CrossAttention Trainium2 kernel.

Full inputs in, full output out. Data-parallel over batch: core b computes
batch item b of 8.

Per-core math (all layouts transposed so the PE contraction dim is always
the partition dim, with no on-chip transposes):
  QT[d, q] = (Wq*scale @ q_b^T)      via lhsT=WqT chunks, rhs=q_b^T
  KT[d, k] = (Wk @ kv_b^T)
  V[k, d]  = (kv_b @ Wv^T)           via lhsT=kv_b^T chunks, rhs=WvT
  S^T[k, q] = K Q^T                  per head, lhsT=KT, rhs=QT (64-contraction,
                                     two heads packed in 128 partitions)
  P^T = exp(S^T - ln256) * exp_posT  (exp_pos precomputed on host, fp16)
  O^T[d, q] (+rowsum row) = V_aug^T-contraction over k: lhsT=V_aug[k,65],
                                     rhs=P^T, psum-accumulated
  X^T = O^T[0:64] * (1/rowsum) broadcast
  out[q, e] = sum_d X^T.T @ WprojT + bias
"""

import numpy as np

B, L, DIM, H, HD = 8, 1024, 768, 12, 64
NCORES = 8
CP = DIM // 128  # 6 chunks of the contraction/feature dim
KC = L // 128    # 8 k-chunks
SCALE = HD ** -0.5
LN_OFF = float(np.log(256.0))

_CACHE = {}


def _build():
    import concourse.bass as bass
    import concourse.mybir as mybir
    import concourse.tile as tile
    from concourse import bacc

    f32 = mybir.dt.float32
    f32r = mybir.dt.float32r
    f16 = mybir.dt.float16
    AF = mybir.ActivationFunctionType

    nc = bacc.Bacc("TRN2", target_bir_lowering=False, debug=False)

    qT = nc.dram_tensor("qT", [DIM, L], f32r, kind="ExternalInput")
    kvT = nc.dram_tensor("kvT", [DIM, L], f32r, kind="ExternalInput")
    wq = nc.dram_tensor("wq", [DIM, DIM], f32r, kind="ExternalInput")  # [c, d]
    wk = nc.dram_tensor("wk", [DIM, DIM], f32r, kind="ExternalInput")  # [c, d]
    wv = nc.dram_tensor("wv", [DIM, DIM], f32r, kind="ExternalInput")  # [c, d]
    wp = nc.dram_tensor("wp", [DIM, DIM], f32r, kind="ExternalInput")  # [d, e]
    bias = nc.dram_tensor("bias", [128, DIM], f32, kind="ExternalInput")
    epos = nc.dram_tensor("epos", [H, L, L], f16, kind="ExternalInput")  # [h,k,q]
    out = nc.dram_tensor("out", [L, DIM], f32, kind="ExternalOutput")
    rscr = nc.dram_tensor("rs_scratch", [H, L], f32)

    with tile.TileContext(nc) as tc:
        with tc.tile_pool(name="persist", bufs=1) as persist:
            QT = persist.tile([128, CP, L], f32r)   # pair p: heads 2p, 2p+1
            KT = persist.tile([128, CP, L], f32r)
            Vt = [
                persist.tile([128, H, HD + 1], f16, name=f"Vt{k}", tag=f"V{k}")
                for k in range(KC)
            ]
            wp_sb = persist.tile([128, CP, DIM], f32r)
            bias_bc = persist.tile([128, DIM], f32)
            rs = persist.tile([38, L], f32)
            recip = persist.tile([38, L], f32)

            nc.sync.dma_start(wp_sb[:], wp.rearrange("(a p) d -> p a d", p=128))
            nc.sync.dma_start(bias_bc[:], bias[:])
            expb = persist.tile([128, 1], f32)
            nc.vector.memset(expb[:], -LN_OFF)

            # ---------------- phase 1: projections ----------------
            with (
                tc.tile_pool(name="ph1", bufs=1) as ph1,
                tc.tile_pool(name="psA", bufs=2, space="PSUM") as psA,
            ):
                q_sb = ph1.tile([128, CP, L], f32r)
                kv_sb = ph1.tile([128, CP, L], f32r)
                wq_sb = ph1.tile([128, CP, DIM], f32r)
                wk_sb = ph1.tile([128, CP, DIM], f32r)
                wv_sb = ph1.tile([128, CP, DIM], f32r)
                nc.sync.dma_start(q_sb[:], qT.rearrange("(a p) q -> p a q", p=128))
                nc.sync.dma_start(kv_sb[:], kvT.rearrange("(a p) q -> p a q", p=128))
                nc.sync.dma_start(wq_sb[:], wq.rearrange("(a p) d -> p a d", p=128))
                nc.sync.dma_start(wk_sb[:], wk.rearrange("(a p) d -> p a d", p=128))
                nc.sync.dma_start(wv_sb[:], wv.rearrange("(a p) d -> p a d", p=128))

                for w_sb, x_sb, dst in ((wq_sb, q_sb, QT), (wk_sb, kv_sb, KT)):
                    for p in range(CP):
                        ps = psA.tile([128, L], f32, tag="proj")
                        for c in range(CP):
                            for hf in range(2):
                                nc.tensor.matmul(
                                    ps[:, hf * 512:(hf + 1) * 512],
                                    w_sb[:, c, p * 128:(p + 1) * 128],
                                    x_sb[:, c, hf * 512:(hf + 1) * 512],
                                    start=(c == 0),
                                    stop=(c == CP - 1),
                                )
                        nc.vector.tensor_copy(dst[:, p, :], ps[:])

                for k in range(KC):
                    ps = psA.tile([128, DIM], f32, tag="proj")
                    for c in range(CP):
                        for lo, sz in ((0, 512), (512, 256)):
                            nc.tensor.matmul(
                                ps[:, lo:lo + sz],
                                kv_sb[:, c, k * 128:(k + 1) * 128],
                                wv_sb[:, c, lo:lo + sz],
                                start=(c == 0),
                                stop=(c == CP - 1),
                            )
                    nc.vector.memset(Vt[k][:, :, HD:HD + 1], 1.0)
                    nc.vector.tensor_copy(
                        Vt[k][:, :, 0:HD],
                        ps[:].rearrange("p (h d) -> p h d", d=HD),
                    )

            # ---------------- phase 2: attention ----------------
            with tc.tile_pool(name="xt", bufs=1) as xtp:
              with (
                tc.tile_pool(name="eposp", bufs=10) as eposp,
                tc.tile_pool(name="praw", bufs=2) as praw,
                tc.tile_pool(name="ptp", bufs=3) as ptp,
                tc.tile_pool(name="xtup", bufs=8) as xtup,
                tc.tile_pool(name="bcp", bufs=2) as bcp,
                tc.tile_pool(name="psS", bufs=2, space="PSUM") as psS,
                tc.tile_pool(name="psO", bufs=2, space="PSUM") as psO,
              ):
                XT = xtp.tile([128, CP, L], f32r)
                xtu = [None] * H

                def normalize(h):
                    p, sub = divmod(h, 2)
                    bc = bcp.tile([64, L], f32, name=f"bc{h}", tag="bc")
                    nc.sync.dma_start(bc[:], rscr[h:h + 1, :].broadcast_to([64, L]))
                    nc.vector.tensor_mul(
                        XT[sub * 64:(sub + 1) * 64, p, :],
                        xtu[h][0:64, :],
                        bc[:],
                    )

                for h in range(H):
                    p, sub = divmod(h, 2)
                    o_ps = psO.tile([65, L], f32)
                    for k in range(KC):
                        s_ps = psS.tile([128, L], f32)
                        for hf in range(2):
                            nc.tensor.matmul(
                                s_ps[:, hf * 512:(hf + 1) * 512],
                                KT[sub * 64:(sub + 1) * 64, p, k * 128:(k + 1) * 128],
                                QT[sub * 64:(sub + 1) * 64, p, hf * 512:(hf + 1) * 512],
                            )
                        pr = praw.tile([128, L], f16)
                        nc.scalar.activation(pr[:], s_ps[:], AF.Exp, bias=expb[:])
                        ep = eposp.tile([128, L], f16)
                        nc.sync.dma_start(ep[:], epos[h, k * 128:(k + 1) * 128, :])
                        pt = ptp.tile([128, L], f16)
                        nc.vector.tensor_mul(pt[:], pr[:], ep[:])
                        for hf in range(2):
                            nc.tensor.matmul(
                                o_ps[:, hf * 512:(hf + 1) * 512],
                                Vt[k][:, h, :],
                                pt[:, hf * 512:(hf + 1) * 512],
                                start=(k == 0),
                                stop=(k == KC - 1),
                            )
                    # evict O^T (incl. rowsum row 64) to SBUF, gather rowsum
                    xtu[h] = xtup.tile([65, L], f32, name=f"xtu{h}", tag="xtu")
                    nc.scalar.copy(xtu[h][:], o_ps[:])
                    row = h if h < 6 else 32 + (h - 6)
                    nc.sync.dma_start(rs[row:row + 1, :], xtu[h][64:65, :])
                    if h == 5:
                        nc.vector.reciprocal(recip[0:6, :], rs[0:6, :])
                        nc.sync.dma_start(rscr[0:6, :], recip[0:6, :])
                        for hh in range(6):
                            normalize(hh)
                    if h == H - 1:
                        nc.vector.reciprocal(recip[32:38, :], rs[32:38, :])
                        nc.sync.dma_start(rscr[6:12, :], recip[32:38, :])
                        for hh in range(6, 12):
                            normalize(hh)

              # ---------------- phase 3: output projection ----------------
              with (
                  tc.tile_pool(name="outp", bufs=2) as outp,
                  tc.tile_pool(name="psOut", bufs=2, space="PSUM") as psOut,
              ):
                  for qc in range(KC):
                      ps = psOut.tile([128, DIM], f32)
                      for d in range(CP):
                          for lo, sz in ((0, 512), (512, 256)):
                              nc.tensor.matmul(
                                  ps[:, lo:lo + sz],
                                  XT[:, d, qc * 128:(qc + 1) * 128],
                                  wp_sb[:, d, lo:lo + sz],
                                  start=(d == 0),
                                  stop=(d == CP - 1),
                              )
                      ot = outp.tile([128, DIM], f32)
                      nc.vector.tensor_add(ot[:], ps[:], bias_bc[:])
                      nc.sync.dma_start(out[qc * 128:(qc + 1) * 128, :], ot[:])

    nc.compile()
    return nc


def _get_nc():
    if "nc" not in _CACHE:
        _CACHE["nc"] = _build()
    return _CACHE["nc"]


def _host_prep(q, kv, attn_pos, Wq, Wkv, Wproj, bproj):
    import ml_dtypes

    q = np.asarray(q, dtype=np.float32)
    kv = np.asarray(kv, dtype=np.float32)
    attn_pos = np.asarray(attn_pos, dtype=np.float32)
    Wq = np.asarray(Wq, dtype=np.float32)
    Wkv = np.asarray(Wkv, dtype=np.float32)
    Wproj = np.asarray(Wproj, dtype=np.float32)
    bproj = np.asarray(bproj, dtype=np.float32)

    wq = np.ascontiguousarray((Wq * SCALE).T)          # [c, d]
    wk = np.ascontiguousarray(Wkv[:DIM].T)             # [c, d]
    wv = np.ascontiguousarray(Wkv[DIM:].T)             # [c, d]
    wp = np.ascontiguousarray(Wproj.T)                 # [d, e]
    bias = np.ascontiguousarray(np.tile(bproj[None, :], (128, 1)))
    # epos[h, k, q] = exp(attn_pos[0, h, q, k])
    epos = np.ascontiguousarray(
        np.exp(attn_pos[0]).transpose(0, 2, 1)
    ).astype(ml_dtypes.float16 if hasattr(ml_dtypes, "float16") else np.float16)
    epos = epos.astype(np.float16)

    qT = np.ascontiguousarray(q.transpose(0, 2, 1))    # [B, c, L]
    kvT = np.ascontiguousarray(kv.transpose(0, 2, 1))  # [B, c, L]

    shared = {"wq": wq, "wk": wk, "wv": wv, "wp": wp, "bias": bias, "epos": epos}
    in_maps = []
    for b in range(B):
        m = dict(shared)
        m["qT"] = qT[b]
        m["kvT"] = kvT[b]
        in_maps.append(m)
    return in_maps


def kernel(q, kv, attn_pos, Wq, Wkv, Wproj, bproj):
    from concourse.bass_utils import run_bass_kernel_spmd

    nc = _get_nc()
    in_maps = _host_prep(q, kv, attn_pos, Wq, Wkv, Wproj, bproj)
    res = run_bass_kernel_spmd(nc, in_maps, list(range(NCORES)))
    return np.stack([res.results[b]["out"] for b in range(B)], axis=0)



# revision 3
# speedup vs baseline: 1.1418x; 1.1418x over previous
"""CrossAttention Trainium2 kernel (v2: fp16 matmul pipeline).

Full inputs in, full output out. Data-parallel over batch: core b computes
batch item b of 8.

Per-core math (layouts transposed so the PE contraction dim is always the
partition dim, no on-chip transposes):
  V[k, d]   = (kv_b @ Wv^T)            8 k-chunks, augmented with a ones col
  QT[d, q]  = (Wq*scale @ q_b^T)       per head-pair p (d = pair dims)
  KT[d, k]  = (Wk @ kv_b^T)
  S^T[k, q] = K Q^T                    per head; the two heads of a pair run
                                       row-tiled (rows 0-63 / 64-127)
  P^T = exp(S^T - ln256) * epos^T      epos precomputed on host, fp16
  O^T[d, q] (+rowsum row 64) = V_aug^T P^T, psum-accumulated over k
  X^T = O^T[0:64] * (1/rowsum)
  out[q, e] = X^T.T @ WprojT + bias
All matmul operands fp16; PSUM accumulation fp32.
"""

import numpy as np

B, L, DIM, H, HD = 8, 1024, 768, 12, 64
NCORES = 8
CP = DIM // 128  # 6 chunks of the contraction/feature dim
KC = L // 128    # 8 k-chunks
NP = H // 2      # 6 head pairs
SCALE = HD ** -0.5
LN_OFF = float(np.log(256.0))

_CACHE = {}


def _build():
    import concourse.bass as bass
    import concourse.mybir as mybir
    import concourse.tile as tile
    from concourse import bacc

    f32 = mybir.dt.float32
    f16 = mybir.dt.float16
    AF = mybir.ActivationFunctionType

    nc = bacc.Bacc("TRN2", target_bir_lowering=False, debug=False)

    qT = nc.dram_tensor("qT", [DIM, L], f16, kind="ExternalInput")
    kvT = nc.dram_tensor("kvT", [DIM, L], f16, kind="ExternalInput")
    wq = nc.dram_tensor("wq", [DIM, DIM], f16, kind="ExternalInput")  # [c, d]
    wk = nc.dram_tensor("wk", [DIM, DIM], f16, kind="ExternalInput")  # [c, d]
    wv = nc.dram_tensor("wv", [DIM, DIM], f16, kind="ExternalInput")  # [c, d]
    wp = nc.dram_tensor("wp", [DIM, DIM], f16, kind="ExternalInput")  # [d, e]
    bias = nc.dram_tensor("bias", [128, DIM], f32, kind="ExternalInput")
    epos = nc.dram_tensor("epos", [H, L, L], f16, kind="ExternalInput")  # [h,k,q]
    out = nc.dram_tensor("out", [L, DIM], f32, kind="ExternalOutput")
    rscr = nc.dram_tensor("rs_scratch", [H, L], f32)

    with tile.TileContext(nc) as tc:
        with tc.tile_pool(name="persist", bufs=1) as persist:
            QT = persist.tile([128, NP, L], f16)   # pair p: heads 2p, 2p+1
            KT = persist.tile([128, NP, L], f16)
            XT = persist.tile([128, NP, L], f16)
            Vt = [
                persist.tile([128, H, HD + 1], f16, name=f"Vt{k}", tag=f"V{k}")
                for k in range(KC)
            ]
            wp_sb = persist.tile([128, CP, DIM], f16)
            bias_bc = persist.tile([128, DIM], f32)
            rs = persist.tile([38, L], f32)
            recip = persist.tile([38, L], f32)
            expb = persist.tile([128, 1], f32)
            nc.vector.memset(expb[:], -LN_OFF)

            q_sb = persist.tile([128, CP, L], f16)
            kv_sb = persist.tile([128, CP, L], f16)
            wq_sb = persist.tile([128, CP, DIM], f16)
            wk_sb = persist.tile([128, CP, DIM], f16)
            wv_sb = persist.tile([128, CP, DIM], f16)

            # input DMAs, chunked so compute can start on the first chunks;
            # kv+wv first (V proj), then q/wq/wk (QK proj), wp/bias last
            kv_r = kvT.rearrange("(a p) q -> p a q", p=128)
            q_r = qT.rearrange("(a p) q -> p a q", p=128)
            wq_r = wq.rearrange("(a p) d -> p a d", p=128)
            wk_r = wk.rearrange("(a p) d -> p a d", p=128)
            wv_r = wv.rearrange("(a p) d -> p a d", p=128)
            wp_r = wp.rearrange("(a p) d -> p a d", p=128)
            for c in range(CP):
                nc.sync.dma_start(kv_sb[:, c, :], kv_r[:, c, :])
                nc.sync.dma_start(wv_sb[:, c, :], wv_r[:, c, :])
            for c in range(CP):
                nc.sync.dma_start(q_sb[:, c, :], q_r[:, c, :])
                nc.sync.dma_start(wq_sb[:, c, :], wq_r[:, c, :])
                nc.sync.dma_start(wk_sb[:, c, :], wk_r[:, c, :])
            for c in range(CP):
                nc.sync.dma_start(wp_sb[:, c, :], wp_r[:, c, :])
            nc.sync.dma_start(bias_bc[:], bias[:])

            with (
                tc.tile_pool(name="psA", bufs=2, space="PSUM") as psA,
                tc.tile_pool(name="psO", bufs=2, space="PSUM") as psO,
                tc.tile_pool(name="eposp", bufs=5) as eposp,
                tc.tile_pool(name="praw", bufs=3) as praw,
                tc.tile_pool(name="ptp", bufs=3) as ptp,
                tc.tile_pool(name="xtup", bufs=8) as xtup,
                tc.tile_pool(name="bcp", bufs=3) as bcp,
            ):
                # ---------------- V projection ----------------
                for k in range(KC):
                    ps = psA.tile([128, L], f32, tag="psA")
                    for c in range(CP):
                        for lo, sz in ((0, 512), (512, 256)):
                            nc.tensor.matmul(
                                ps[:, lo:lo + sz],
                                kv_sb[:, c, k * 128:(k + 1) * 128],
                                wv_sb[:, c, lo:lo + sz],
                                start=(c == 0),
                                stop=(c == CP - 1),
                            )
                    nc.vector.memset(Vt[k][:, :, HD:HD + 1], 1.0)
                    nc.vector.tensor_copy(
                        Vt[k][:, :, 0:HD],
                        ps[:, 0:DIM].rearrange("p (h d) -> p h d", d=HD),
                    )

                # ---------------- QK projection for one pair ----------------
                def project_pair(p):
                    for w_sb, x_sb, dst in ((wq_sb, q_sb, QT), (wk_sb, kv_sb, KT)):
                        ps = psA.tile([128, L], f32, tag="psA")
                        for c in range(CP):
                            for hf in range(2):
                                nc.tensor.matmul(
                                    ps[:, hf * 512:(hf + 1) * 512],
                                    w_sb[:, c, p * 128:(p + 1) * 128],
                                    x_sb[:, c, hf * 512:(hf + 1) * 512],
                                    start=(c == 0),
                                    stop=(c == CP - 1),
                                )
                        nc.vector.tensor_copy(dst[:, p, :], ps[:])

                project_pair(0)

                xtu = [None] * H

                def normalize(h):
                    p, sub = divmod(h, 2)
                    bc = bcp.tile([64, L], f32, name=f"bc{h}", tag="bc")
                    nc.sync.dma_start(bc[:], rscr[h:h + 1, :].broadcast_to([64, L]))
                    nc.vector.tensor_mul(
                        XT[sub * 64:(sub + 1) * 64, p, :],
                        xtu[h][0:64, :],
                        bc[:],
                    )

                # ---------------- attention, pair at a time ----------------
                for p in range(NP):
                    h0, h1 = 2 * p, 2 * p + 1
                    o_ps0 = psO.tile([HD + 1, L], f32, tag="psO")
                    o_ps1 = psO.tile([HD + 1, L], f32, tag="psO")
                    for k in range(KC):
                        s0 = psA.tile([128, L], f32, tag="psA")
                        s1 = psA.tile([128, L], f32, tag="psA")
                        kt_sl = slice(k * 128, (k + 1) * 128)
                        for hf in range(2):
                            qs = slice(hf * 512, (hf + 1) * 512)
                            nc.tensor.matmul(
                                s0[:, qs], KT[0:64, p, kt_sl], QT[0:64, p, qs],
                            )
                            nc.tensor.matmul(
                                s1[:, qs], KT[64:128, p, kt_sl], QT[64:128, p, qs],
                            )
                        for h, s_ps in ((h0, s0), (h1, s1)):
                            pr = praw.tile([128, L], f16, tag="pr")
                            nc.scalar.activation(pr[:], s_ps[:], AF.Exp, bias=expb[:])
                            ep = eposp.tile([128, L], f16, tag="ep")
                            nc.sync.dma_start(ep[:], epos[h, kt_sl, :])
                            pt = ptp.tile([128, L], f16, tag="pt")
                            nc.vector.tensor_mul(pt[:], pr[:], ep[:])
                            o_ps = o_ps0 if h == h0 else o_ps1
                            for hf in range(2):
                                qs = slice(hf * 512, (hf + 1) * 512)
                                nc.tensor.matmul(
                                    o_ps[:, qs],
                                    Vt[k][:, h, :],
                                    pt[:, qs],
                                    start=(k == 0),
                                    stop=(k == KC - 1),
                                )
                    for h, o_ps in ((h0, o_ps0), (h1, o_ps1)):
                        xtu[h] = xtup.tile([HD + 1, L], f32, name=f"xtu{h}", tag="xtu")
                        nc.vector.tensor_copy(xtu[h][:], o_ps[:])
                        row = h if h < 6 else 32 + (h - 6)
                        nc.sync.dma_start(rs[row:row + 1, :], xtu[h][HD:HD + 1, :])
                    if p == 2:
                        nc.vector.reciprocal(recip[0:6, :], rs[0:6, :])
                        nc.sync.dma_start(rscr[0:6, :], recip[0:6, :])
                        for hh in range(6):
                            normalize(hh)
                    if p == NP - 1:
                        nc.vector.reciprocal(recip[32:38, :], rs[32:38, :])
                        nc.sync.dma_start(rscr[6:12, :], recip[32:38, :])
                        for hh in range(6, 12):
                            normalize(hh)
                    if p + 1 < NP:
                        project_pair(p + 1)

            # ---------------- output projection ----------------
            with (
                tc.tile_pool(name="outp", bufs=2) as outp,
                tc.tile_pool(name="psOut", bufs=2, space="PSUM") as psOut,
            ):
                for qc in range(KC):
                    ps = psOut.tile([128, DIM], f32)
                    for d in range(CP):
                        for lo, sz in ((0, 512), (512, 256)):
                            nc.tensor.matmul(
                                ps[:, lo:lo + sz],
                                XT[:, d, qc * 128:(qc + 1) * 128],
                                wp_sb[:, d, lo:lo + sz],
                                start=(d == 0),
                                stop=(d == CP - 1),
                            )
                    ot = outp.tile([128, DIM], f32)
                    nc.vector.tensor_add(ot[:], ps[:], bias_bc[:])
                    nc.sync.dma_start(out[qc * 128:(qc + 1) * 128, :], ot[:])

    nc.compile()
    return nc


def _get_nc():
    if "nc" not in _CACHE:
        _CACHE["nc"] = _build()
    return _CACHE["nc"]


def _host_prep(q, kv, attn_pos, Wq, Wkv, Wproj, bproj):
    q = np.asarray(q, dtype=np.float32)
    kv = np.asarray(kv, dtype=np.float32)
    attn_pos = np.asarray(attn_pos, dtype=np.float32)
    Wq = np.asarray(Wq, dtype=np.float32)
    Wkv = np.asarray(Wkv, dtype=np.float32)
    Wproj = np.asarray(Wproj, dtype=np.float32)
    bproj = np.asarray(bproj, dtype=np.float32)

    wq16 = np.ascontiguousarray((Wq * SCALE).T).astype(np.float16)   # [c, d]
    wk16 = np.ascontiguousarray(Wkv[:DIM].T).astype(np.float16)      # [c, d]
    wv16 = np.ascontiguousarray(Wkv[DIM:].T).astype(np.float16)      # [c, d]
    wp16 = np.ascontiguousarray(Wproj.T).astype(np.float16)          # [d, e]
    bias = np.ascontiguousarray(np.tile(bproj[None, :], (128, 1)))
    # epos[h, k, q] = exp(attn_pos[0, h, q, k])
    epos = np.ascontiguousarray(
        np.exp(attn_pos[0]).transpose(0, 2, 1)
    ).astype(np.float16)

    qT = np.ascontiguousarray(q.transpose(0, 2, 1)).astype(np.float16)
    kvT = np.ascontiguousarray(kv.transpose(0, 2, 1)).astype(np.float16)

    shared = {
        "wq": wq16, "wk": wk16, "wv": wv16, "wp": wp16,
        "bias": bias, "epos": epos,
    }
    in_maps = []
    for b in range(B):
        m = dict(shared)
        m["qT"] = qT[b]
        m["kvT"] = kvT[b]
        in_maps.append(m)
    return in_maps


def kernel(q, kv, attn_pos, Wq, Wkv, Wproj, bproj):
    from concourse.bass_utils import run_bass_kernel_spmd

    nc = _get_nc()
    in_maps = _host_prep(q, kv, attn_pos, Wq, Wkv, Wproj, bproj)
    res = run_bass_kernel_spmd(nc, in_maps, list(range(NCORES)))
    return np.stack([res.results[b]["out"] for b in range(B)], axis=0)


# revision 5
# speedup vs baseline: 1.4880x; 1.3032x over previous
"""CrossAttention Trainium2 kernel (v3: fp16 matmuls, HAM-warm PE stream).

Full inputs in, full output out. Data-parallel over batch: core b computes
batch item b of 8.

Per-core math (layouts transposed so the PE contraction dim is always the
partition dim, no on-chip transposes):
  V[k, d]   = (kv_b @ Wv^T)            8 k-chunks, augmented with a ones col
  QT[d, q]  = (Wq*scale @ q_b^T)       per head-pair p (d = pair dims)
  KT[d, k]  = (Wk @ kv_b^T)
  S^T[k, q] = K Q^T                    per head; the two heads of a pair run
                                       row-tiled (rows 0-63 / 64-127)
  P^T = exp(S^T - ln256) * epos^T      epos precomputed on host, bf16
  O^T[d, q] (+rowsum row 64) = V_aug^T P^T, psum-accumulated over k
  X^T = O^T[0:64] * (1/rowsum)
  out[q, e] = X^T.T @ WprojT + bias

The QK projections for pair p+1 are emitted inside pair p's k-loop so the
PE always has independent matmuls to fill softmax-chain gaps (keeps the
HAM clock gate at 8/8). A warmup matmul burst covers the initial DMA wait.
"""

import numpy as np

B, L, DIM, H, HD = 8, 1024, 768, 12, 64
NCORES = 8
CP = DIM // 128  # 6 chunks of the contraction/feature dim
KC = L // 128    # 8 k-chunks
NP = H // 2      # 6 head pairs
SCALE = HD ** -0.5
LN_OFF = float(np.log(256.0))

_CACHE = {}


def _build():
    import concourse.bass as bass
    import concourse.mybir as mybir
    import concourse.tile as tile
    from concourse import bacc

    f32 = mybir.dt.float32
    f16 = mybir.dt.float16
    bf16 = mybir.dt.bfloat16
    AF = mybir.ActivationFunctionType

    nc = bacc.Bacc("TRN2", target_bir_lowering=False, debug=False)

    qT = nc.dram_tensor("qT", [DIM, L], f16, kind="ExternalInput")
    kvT = nc.dram_tensor("kvT", [DIM, L], f16, kind="ExternalInput")
    wq = nc.dram_tensor("wq", [DIM, DIM], f16, kind="ExternalInput")  # [c, d]
    wk = nc.dram_tensor("wk", [DIM, DIM], f16, kind="ExternalInput")  # [c, d]
    wv = nc.dram_tensor("wv", [DIM, DIM], f16, kind="ExternalInput")  # [c, d]
    wp = nc.dram_tensor("wp", [DIM, DIM], f16, kind="ExternalInput")  # [d, e]
    bias = nc.dram_tensor("bias", [128, DIM], f32, kind="ExternalInput")
    epos = nc.dram_tensor("epos", [H, L, L], bf16, kind="ExternalInput")  # [h,k,q]
    out = nc.dram_tensor("out", [L, DIM], f32, kind="ExternalOutput")
    rscr = nc.dram_tensor("rs_scratch", [H, L], f32)

    with tile.TileContext(nc) as tc:
        with tc.tile_pool(name="persist", bufs=1) as persist:
            QT = persist.tile([128, NP, L], f16)   # pair p: heads 2p, 2p+1
            KT = persist.tile([128, NP, L], f16)
            XT = persist.tile([128, NP, L], f16)
            Vt = [
                persist.tile([128, H, HD + 1], f16, name=f"Vt{k}", tag=f"V{k}")
                for k in range(KC)
            ]
            wp_sb = persist.tile([128, CP, DIM], f16)
            bias_bc = persist.tile([128, DIM], f32)
            rs_a = persist.tile([6, L], f32)
            rs_b = persist.tile([6, L], f32)
            recip_a = persist.tile([6, L], f32)
            recip_b = persist.tile([6, L], f32)
            expb = persist.tile([128, 1], f32)
            nc.vector.memset(expb[:], -LN_OFF)
            warm_w = persist.tile([128, 128], f16)
            warm_x = persist.tile([128, 512], f16)
            nc.vector.memset(warm_w[:], 0.0)
            nc.vector.memset(warm_x[:], 0.0)

            q_sb = persist.tile([128, CP, L], f16)
            kv_sb = persist.tile([128, CP, L], f16)
            wq_sb = persist.tile([128, CP, DIM], f16)
            wk_sb = persist.tile([128, CP, DIM], f16)
            wv_sb = persist.tile([128, CP, DIM], f16)

            # input DMAs, chunked so compute can start on the first chunks;
            # kv+wv first (V proj), then q/wq/wk (QK proj), wp/bias last
            kv_r = kvT.rearrange("(a p) q -> p a q", p=128)
            q_r = qT.rearrange("(a p) q -> p a q", p=128)
            wq_r = wq.rearrange("(a p) d -> p a d", p=128)
            wk_r = wk.rearrange("(a p) d -> p a d", p=128)
            wv_r = wv.rearrange("(a p) d -> p a d", p=128)
            wp_r = wp.rearrange("(a p) d -> p a d", p=128)
            for c in range(CP):
                nc.sync.dma_start(kv_sb[:, c, :], kv_r[:, c, :])
                nc.sync.dma_start(wv_sb[:, c, :], wv_r[:, c, :])
            for c in range(CP):
                nc.sync.dma_start(q_sb[:, c, :], q_r[:, c, :])
                nc.sync.dma_start(wq_sb[:, c, :], wq_r[:, c, :])
                nc.sync.dma_start(wk_sb[:, c, :], wk_r[:, c, :])
            for c in range(CP):
                nc.sync.dma_start(wp_sb[:, c, :], wp_r[:, c, :])
            nc.sync.dma_start(bias_bc[:], bias[:])

            with (
                tc.tile_pool(name="psA", bufs=2, space="PSUM") as psA,
                tc.tile_pool(name="psO", bufs=2, space="PSUM") as psO,
                tc.tile_pool(name="eposp", bufs=5) as eposp,
                tc.tile_pool(name="praw", bufs=3) as praw,
                tc.tile_pool(name="ptp", bufs=3) as ptp,
                tc.tile_pool(name="xtup", bufs=8) as xtup,
                tc.tile_pool(name="bcp", bufs=3) as bcp,
            ):
                # warmup burst: dense matmuls on zeros during the initial
                # input DMA so the PE HAM gate is at 8/8 when real work lands
                wps = psA.tile([128, 512], f32, tag="psA")
                for _ in range(16):
                    nc.tensor.matmul(wps[:], warm_w[:], warm_x[:])

                # ---------------- V projection ----------------
                for k in range(KC):
                    ps = psA.tile([128, L], f32, tag="psA")
                    for c in range(CP):
                        for lo, sz in ((0, 512), (512, 256)):
                            nc.tensor.matmul(
                                ps[:, lo:lo + sz],
                                kv_sb[:, c, k * 128:(k + 1) * 128],
                                wv_sb[:, c, lo:lo + sz],
                                start=(c == 0),
                                stop=(c == CP - 1),
                            )
                    nc.vector.memset(Vt[k][:, :, HD:HD + 1], 1.0)
                    nc.vector.tensor_copy(
                        Vt[k][:, :, 0:HD],
                        ps[:, 0:DIM].rearrange("p (h d) -> p h d", d=HD),
                    )

                # QK projection for pair 0 (rest interleave with attention)
                for w_sb, x_sb, dst in ((wq_sb, q_sb, QT), (wk_sb, kv_sb, KT)):
                    ps = psA.tile([128, L], f32, tag="psA")
                    for c in range(CP):
                        for hf in range(2):
                            nc.tensor.matmul(
                                ps[:, hf * 512:(hf + 1) * 512],
                                w_sb[:, c, 0:128],
                                x_sb[:, c, hf * 512:(hf + 1) * 512],
                                start=(c == 0),
                                stop=(c == CP - 1),
                            )
                    nc.vector.tensor_copy(dst[:, 0, :], ps[:])

                xtu = [None] * H

                def normalize(h, recip_t):
                    p, sub = divmod(h, 2)
                    bc = bcp.tile([64, L], f32, name=f"bc{h}", tag="bc")
                    nc.sync.dma_start(bc[:], rscr[h:h + 1, :].broadcast_to([64, L]))
                    nc.vector.tensor_mul(
                        XT[sub * 64:(sub + 1) * 64, p, :],
                        xtu[h][0:64, :],
                        bc[:],
                    )

                # ---------------- attention, pair at a time ----------------
                for p in range(NP):
                    h0, h1 = 2 * p, 2 * p + 1
                    o_ps0 = psO.tile([HD + 1, L], f32, tag="psO")
                    o_ps1 = psO.tile([HD + 1, L], f32, tag="psO")
                    # projection work for pair p+1, fed into the k-loop as
                    # PE gap filler: list of (psum, wmat, xmat, c, hf)
                    proj_jobs = []
                    if p + 1 < NP:
                        qps = psA.tile([128, L], f32, tag="psA", name=f"qp{p}")
                        kps = psA.tile([128, L], f32, tag="psA", name=f"kp{p}")
                        for ps_t, w_sb, x_sb in (
                            (qps, wq_sb, q_sb), (kps, wk_sb, kv_sb),
                        ):
                            for c in range(CP):
                                for hf in range(2):
                                    proj_jobs.append((ps_t, w_sb, x_sb, c, hf))
                    nj = 0

                    def drain_proj(n):
                        nonlocal nj
                        for _ in range(n):
                            if nj >= len(proj_jobs):
                                return
                            ps_t, w_sb, x_sb, c, hf = proj_jobs[nj]
                            nc.tensor.matmul(
                                ps_t[:, hf * 512:(hf + 1) * 512],
                                w_sb[:, c, (p + 1) * 128:(p + 2) * 128],
                                x_sb[:, c, hf * 512:(hf + 1) * 512],
                                start=(c == 0),
                                stop=(c == CP - 1),
                            )
                            nj += 1

                    for k in range(KC):
                        s0 = psA.tile([128, L], f32, tag="psA")
                        s1 = psA.tile([128, L], f32, tag="psA")
                        kt_sl = slice(k * 128, (k + 1) * 128)
                        for hf in range(2):
                            qs = slice(hf * 512, (hf + 1) * 512)
                            nc.tensor.matmul(
                                s0[:, qs], KT[0:64, p, kt_sl], QT[0:64, p, qs],
                            )
                            nc.tensor.matmul(
                                s1[:, qs], KT[64:128, p, kt_sl], QT[64:128, p, qs],
                            )
                        drain_proj(2)
                        for h, s_ps in ((h0, s0), (h1, s1)):
                            pr = praw.tile([128, L], bf16, tag="pr")
                            nc.scalar.activation(pr[:], s_ps[:], AF.Exp, bias=expb[:])
                            ep = eposp.tile([128, L], bf16, tag="ep")
                            nc.sync.dma_start(ep[:], epos[h, kt_sl, :])
                            pt = ptp.tile([128, L], bf16, tag="pt")
                            nc.vector.tensor_mul(pt[:], pr[:], ep[:])
                            o_ps = o_ps0 if h == h0 else o_ps1
                            for hf in range(2):
                                qs = slice(hf * 512, (hf + 1) * 512)
                                nc.tensor.matmul(
                                    o_ps[:, qs],
                                    Vt[k][:, h, :],
                                    pt[:, qs],
                                    start=(k == 0),
                                    stop=(k == KC - 1),
                                )
                            drain_proj(1)
                        if k == 3 and proj_jobs:
                            nc.vector.tensor_copy(QT[:, p + 1, :], qps[:])
                    if proj_jobs:
                        drain_proj(len(proj_jobs))
                        nc.vector.tensor_copy(KT[:, p + 1, :], kps[:])
                    for h, o_ps in ((h0, o_ps0), (h1, o_ps1)):
                        xtu[h] = xtup.tile([HD + 1, L], f32, name=f"xtu{h}", tag="xtu")
                        nc.vector.tensor_copy(xtu[h][:], o_ps[:])
                        rs_t = rs_a if h < 6 else rs_b
                        nc.sync.dma_start(
                            rs_t[h % 6:h % 6 + 1, :], xtu[h][HD:HD + 1, :]
                        )
                    if p == 2:
                        nc.vector.reciprocal_approx_fast(recip_a[:], rs_a[:])
                        nc.sync.dma_start(rscr[0:6, :], recip_a[:])
                        for hh in range(6):
                            normalize(hh, recip_a)
                    if p == NP - 1:
                        nc.vector.reciprocal_approx_fast(recip_b[:], rs_b[:])
                        nc.sync.dma_start(rscr[6:12, :], recip_b[:])
                        for hh in range(6, 12):
                            normalize(hh, recip_b)

            # ---------------- output projection ----------------
            with (
                tc.tile_pool(name="outp", bufs=2) as outp,
                tc.tile_pool(name="psOut", bufs=2, space="PSUM") as psOut,
            ):
                for qc in range(KC):
                    ps = psOut.tile([128, DIM], f32)
                    for d in range(CP):
                        for lo, sz in ((0, 512), (512, 256)):
                            nc.tensor.matmul(
                                ps[:, lo:lo + sz],
                                XT[:, d, qc * 128:(qc + 1) * 128],
                                wp_sb[:, d, lo:lo + sz],
                                start=(d == 0),
                                stop=(d == CP - 1),
                            )
                    ot = outp.tile([128, DIM], f32)
                    nc.vector.tensor_add(ot[:], ps[:], bias_bc[:])
                    nc.sync.dma_start(out[qc * 128:(qc + 1) * 128, :], ot[:])

    nc.compile()
    return nc


def _get_nc():
    if "nc" not in _CACHE:
        _CACHE["nc"] = _build()
    return _CACHE["nc"]


def _host_prep(q, kv, attn_pos, Wq, Wkv, Wproj, bproj):
    import ml_dtypes

    q = np.asarray(q, dtype=np.float32)
    kv = np.asarray(kv, dtype=np.float32)
    attn_pos = np.asarray(attn_pos, dtype=np.float32)
    Wq = np.asarray(Wq, dtype=np.float32)
    Wkv = np.asarray(Wkv, dtype=np.float32)
    Wproj = np.asarray(Wproj, dtype=np.float32)
    bproj = np.asarray(bproj, dtype=np.float32)

    wq16 = np.ascontiguousarray((Wq * SCALE).T).astype(np.float16)   # [c, d]
    wk16 = np.ascontiguousarray(Wkv[:DIM].T).astype(np.float16)      # [c, d]
    wv16 = np.ascontiguousarray(Wkv[DIM:].T).astype(np.float16)      # [c, d]
    wp16 = np.ascontiguousarray(Wproj.T).astype(np.float16)          # [d, e]
    bias = np.ascontiguousarray(np.tile(bproj[None, :], (128, 1)))
    # epos[h, k, q] = exp(attn_pos[0, h, q, k])
    epos = np.ascontiguousarray(
        np.exp(attn_pos[0]).transpose(0, 2, 1)
    ).astype(ml_dtypes.bfloat16)

    qT = np.ascontiguousarray(q.transpose(0, 2, 1)).astype(np.float16)
    kvT = np.ascontiguousarray(kv.transpose(0, 2, 1)).astype(np.float16)

    shared = {
        "wq": wq16, "wk": wk16, "wv": wv16, "wp": wp16,
        "bias": bias, "epos": epos,
    }
    in_maps = []
    for b in range(B):
        m = dict(shared)
        m["qT"] = qT[b]
        m["kvT"] = kvT[b]
        in_maps.append(m)
    return in_maps


def kernel(q, kv, attn_pos, Wq, Wkv, Wproj, bproj):
    from concourse.bass_utils import run_bass_kernel_spmd

    nc = _get_nc()
    in_maps = _host_prep(q, kv, attn_pos, Wq, Wkv, Wproj, bproj)
    res = run_bass_kernel_spmd(nc, in_maps, list(range(NCORES)))
    return np.stack([res.results[b]["out"] for b in range(B)], axis=0)
